# Initial kernel scaffold
#
"""Trainium2 Bass kernel for nn_AugmentWithTrace (gnn_message_passing).

Reference computation:
    g = trace_pool[neighbor_idx]                       # [T, K, D] gather
    s = MLP3(g)                                        # per-row scores
    attn = masked_softmax_k(s)                         # over K=8 neighbors
    out = einsum('tk,tkd->td', attn, g)                # [T, D]

Sharding: data-parallel over T across 8 NeuronCores; trace_pool and MLP
weights replicated. Per core: 4096 tokens = 32 chunks of 128 tokens.

Per-chunk dataflow (per core):
  - 8 indirect row-gathers (one per neighbor slot k): G[:, k*256:(k+1)*256]
    holds pool rows for neighbor k of the 128 tokens (token t -> partition t).
  - cast G to bf16, one HWDGE DMA-transpose -> XT [128, 16, 128] where
    slot 2k+h = (G_k columns h*128..h*128+127)^T, i.e. feature f of neighbor k
    lives at partition f%128, slot 2k + f//128.
  - 3-layer MLP in bf16 on the tensor engine (f32 PSUM accumulate), ReLU+bias
    fused into scalar-engine activation epilogues.
  - scores [1, 1024] transposed to [128 tokens, 8 k] via tiny K=1 matmuls.
  - no-max softmax: exp on ACT, exact-zero masking + sum via one DVE
    tensor_tensor_reduce, reciprocal, scale.
  - weighted sum: 8 fused multiply-add DVE ops against the f32 G tiles.
"""

import sys

if "/opt/trn_rl_repo" not in sys.path:
    sys.path.insert(0, "/opt/trn_rl_repo")

import numpy as np
import ml_dtypes

T, K, D, N_POOL = 32768, 8, 256, 131072
N_CORES = 8
T_LOC = T // N_CORES          # 4096 tokens per core
N_CHUNK = T_LOC // 128        # 32 chunks of 128 tokens

_CACHE = {}


def _build_kernel():
    import concourse.bass as bass
    import concourse.bacc as bacc
    import concourse.mybir as mybir
    import concourse.tile as tile

    f32 = mybir.dt.float32
    bf16 = mybir.dt.bfloat16
    i32 = mybir.dt.int32

    nc = bacc.Bacc("TRN2", target_bir_lowering=False, debug=False,
                   num_devices=N_CORES)

    pool_d = nc.declare_dram_parameter("pool", [N_POOL, D], f32, isOutput=False)
    idx_d = nc.declare_dram_parameter("idx", [128, N_CHUNK * K], i32, isOutput=False)
    maskc_d = nc.declare_dram_parameter("maskc", [128, N_CHUNK * K], f32, isOutput=False)
    w1_d = nc.declare_dram_parameter("w1", [128, 512], bf16, isOutput=False)
    w2_d = nc.declare_dram_parameter("w2", [128, 512], bf16, isOutput=False)
    w3_d = nc.declare_dram_parameter("w3", [128, 2], bf16, isOutput=False)
    b1_d = nc.declare_dram_parameter("b1c", [128, 2], f32, isOutput=False)
    b2_d = nc.declare_dram_parameter("b2c", [128, 2], f32, isOutput=False)
    b3_d = nc.declare_dram_parameter("b3v", [1, 1], f32, isOutput=False)
    out_d = nc.declare_dram_parameter("out", [T_LOC, D], f32, isOutput=True)

    with tile.TileContext(nc) as tc:
        with (
            tc.tile_pool(name="const", bufs=1) as cp,
            tc.tile_pool(name="gat", bufs=3) as gp,
            tc.tile_pool(name="gbf", bufs=2) as gbp,
            tc.tile_pool(name="xt", bufs=2) as xp,
            tc.tile_pool(name="act", bufs=2) as hp,
            tc.tile_pool(name="sml", bufs=2) as sp,
            tc.tile_pool(name="outp", bufs=2) as op_,
            tc.tile_pool(name="pmm", bufs=2, space="PSUM") as pmm,
            tc.tile_pool(name="psml", bufs=1, space="PSUM") as psml,
            tc.tile_pool(name="psc", bufs=2, space="PSUM") as psc,
        ):
            # ---- constants loaded once ----
            idx_t = cp.tile([128, N_CHUNK * K], i32)
            nc.sync.dma_start(out=idx_t[:], in_=idx_d[:])
            maskc_t = cp.tile([128, N_CHUNK * K], f32)
            nc.sync.dma_start(out=maskc_t[:], in_=maskc_d[:])
            w1_t = cp.tile([128, 512], bf16)
            nc.sync.dma_start(out=w1_t[:], in_=w1_d[:])
            w2_t = cp.tile([128, 512], bf16)
            nc.sync.dma_start(out=w2_t[:], in_=w2_d[:])
            w3_t = cp.tile([128, 2], bf16)
            nc.sync.dma_start(out=w3_t[:], in_=w3_d[:])
            b1_t = cp.tile([128, 2], f32)
            nc.sync.dma_start(out=b1_t[:], in_=b1_d[:])
            b2_t = cp.tile([128, 2], f32)
            nc.sync.dma_start(out=b2_t[:], in_=b2_d[:])
            b3_t = cp.tile([1, 1], f32)
            nc.sync.dma_start(out=b3_t[:], in_=b3_d[:])
            ones1 = cp.tile([1, 1], bf16)
            nc.vector.memset(ones1[:], 1.0)

            relu = mybir.ActivationFunctionType.Relu
            expf = mybir.ActivationFunctionType.Exp

            for c in range(N_CHUNK):
                # ---- gather: 8 indirect 128-row calls ----
                g_t = gp.tile([128, K * D], f32, name=f"g{c}", tag="g")
                for k in range(K):
                    nc.gpsimd.indirect_dma_start(
                        out=g_t[:, k * D:(k + 1) * D],
                        out_offset=None,
                        in_=pool_d[:],
                        in_offset=bass.IndirectOffsetOnAxis(
                            ap=idx_t[:, c * K + k:c * K + k + 1], axis=0),
                    )

                # ---- bf16 cast + feature transpose ----
                gb_t = gbp.tile([128, K * D], bf16, name=f"gb{c}", tag="gb")
                nc.vector.tensor_copy(out=gb_t[:], in_=g_t[:])
                xt_t = xp.tile([128, 16 * 128], bf16, name=f"xt{c}", tag="xt")
                nc.sync.dma_start_transpose(
                    out=xt_t[:].rearrange("p (s c) -> p s c", s=16),
                    in_=gb_t[:])

                # layer helper views: slot layout (k, h) with slot = 2k+h
                def half(tile_ap, h):
                    # [128, 8, 128]: the h-th 128-feature half of all 8 k's
                    return tile_ap.rearrange(
                        "p (kk hh c) -> p kk hh c", kk=8, hh=2)[:, :, h, :]

                # ---- layer 1 ----
                h1_t = hp.tile([128, 16 * 128], bf16, name=f"h1{c}", tag="h1")
                for j in range(2):
                    ps1 = pmm.tile([128, 1024], f32, name=f"ps1{c}{j}", tag="mm")
                    xv = xt_t[:].rearrange("p (s c) -> p s c", s=16)
                    for h in range(2):
                        for n4 in range(2):
                            nc.tensor.matmul(
                                out=ps1[:].rearrange("p (a c) -> p a c", a=8)
                                    [:, n4 * 4:(n4 + 1) * 4, :],
                                lhsT=w1_t[:, h * 256 + j * 128:h * 256 + (j + 1) * 128],
                                rhs=half(xv, h)[:, n4 * 4:(n4 + 1) * 4, :],
                                start=(h == 0), stop=(h == 1),
                            )
                    nc.scalar.activation(
                        out=h1_t[:].rearrange("p (kk hh c) -> p hh (kk c)", hh=2)
                            [:, j:j + 1, :],
                        in_=ps1[:].rearrange("p (a c) -> p a c", a=1),
                        func=relu, bias=b1_t[:, j:j + 1], scale=1.0)

                # ---- layer 2 ----
                h2_t = hp.tile([128, 16 * 128], bf16, name=f"h2{c}", tag="h2")
                for j in range(2):
                    ps2 = pmm.tile([128, 1024], f32, name=f"ps2{c}{j}", tag="mm")
                    hv = h1_t[:].rearrange("p (s c) -> p s c", s=16)
                    for h in range(2):
                        for n4 in range(2):
                            nc.tensor.matmul(
                                out=ps2[:].rearrange("p (a c) -> p a c", a=8)
                                    [:, n4 * 4:(n4 + 1) * 4, :],
                                lhsT=w2_t[:, h * 256 + j * 128:h * 256 + (j + 1) * 128],
                                rhs=half(hv, h)[:, n4 * 4:(n4 + 1) * 4, :],
                                start=(h == 0), stop=(h == 1),
                            )
                    nc.scalar.activation(
                        out=h2_t[:].rearrange("p (kk hh c) -> p hh (kk c)", hh=2)
                            [:, j:j + 1, :],
                        in_=ps2[:].rearrange("p (a c) -> p a c", a=1),
                        func=relu, bias=b2_t[:, j:j + 1], scale=1.0)

                # ---- layer 3: scores [1, 1024] ----
                ps_s = psml.tile([1, 1024], f32, name=f"pss{c}", tag="ps")
                h2v = h2_t[:].rearrange("p (s c) -> p s c", s=16)
                for n4 in range(2):
                    for h in range(2):
                        nc.tensor.matmul(
                            out=ps_s[:, n4 * 512:(n4 + 1) * 512],
                            lhsT=w3_t[:, h:h + 1],
                            rhs=half(h2v, h)[:, n4 * 4:(n4 + 1) * 4, :],
                            start=(h == 0), stop=(h == 1),
                        )
                s_t = sp.tile([1, 1024], bf16, name=f"s{c}", tag="s")
                nc.vector.tensor_copy(out=s_t[:], in_=ps_s[:])

                # ---- transpose scores to [128 tokens, 8 k] ----
                ps_c = psc.tile([128, K], f32, name=f"psc{c}", tag="sc")
                for k in range(K):
                    nc.tensor.matmul(
                        out=ps_c[:, k:k + 1],
                        lhsT=s_t[:, k * 128:(k + 1) * 128],
                        rhs=ones1[:],
                        start=True, stop=True,
                    )

                # ---- softmax over k (no max-subtraction; exact-0 masking) ----
                e_t = sp.tile([128, K], f32, name=f"e{c}", tag="e")
                nc.scalar.activation(out=e_t[:], in_=ps_c[:], func=expf,
                                     bias=b3_t[:1, :].to_broadcast([128, 1]),
                                     scale=1.0)
                em_t = sp.tile([128, K], f32, name=f"em{c}", tag="em")
                z_t = sp.tile([128, 1], f32, name=f"z{c}", tag="z")
                nc.vector.tensor_tensor_reduce(
                    out=em_t[:], in0=e_t[:],
                    in1=maskc_t[:, c * K:(c + 1) * K],
                    scale=1.0, scalar=1e-30,
                    op0=mybir.AluOpType.mult, op1=mybir.AluOpType.add,
                    accum_out=z_t[:])
                r_t = sp.tile([128, 1], f32, name=f"r{c}", tag="r")
                nc.vector.reciprocal(out=r_t[:], in_=z_t[:])
                a_t = sp.tile([128, K], f32, name=f"a{c}", tag="a")
                nc.vector.tensor_scalar_mul(a_t[:], em_t[:], r_t[:])

                # ---- weighted sum over k ----
                acc_t = op_.tile([128, D], f32, name=f"acc{c}", tag="acc")
                nc.vector.tensor_scalar_mul(
                    acc_t[:], g_t[:, 0:D], a_t[:, 0:1])
                for k in range(1, K):
                    nc.vector.scalar_tensor_tensor(
                        out=acc_t[:],
                        in0=g_t[:, k * D:(k + 1) * D],
                        scalar=a_t[:, k:k + 1],
                        in1=acc_t[:],
                        op0=mybir.AluOpType.mult,
                        op1=mybir.AluOpType.add)

                nc.sync.dma_start(out=out_d[c * 128:(c + 1) * 128, :],
                                  in_=acc_t[:])

    nc.compile()
    return nc


def _prep_core_inputs(c, pool_f32, neighbor_idx, mask_f, w_shared):
    t0 = c * T_LOC
    nidx = neighbor_idx[t0:t0 + T_LOC]                     # [T_LOC, K]
    # column (chunk*K + k), partition p -> token chunk*128+p, neighbor k
    idx_arr = np.ascontiguousarray(
        nidx.reshape(N_CHUNK, 128, K).transpose(1, 0, 2).reshape(128, N_CHUNK * K)
    ).astype(np.int32)
    mcol = np.ascontiguousarray(
        mask_f[t0:t0 + T_LOC].reshape(N_CHUNK, 128, K)
        .transpose(1, 0, 2).reshape(128, N_CHUNK * K)
    ).astype(np.float32)
    m = {"pool": pool_f32, "idx": idx_arr, "maskc": mcol}
    m.update(w_shared)
    return m


def kernel(trace_pool, neighbor_idx, neighbor_mask, W1, b1, W2, b2, W3, b3):
    if "nc" not in _CACHE:
        _CACHE["nc"] = _build_kernel()
    nc = _CACHE["nc"]

    from concourse.bass_utils import run_bass_kernel_spmd

    pool_f32 = np.ascontiguousarray(np.asarray(trace_pool, dtype=np.float32))
    neighbor_idx = np.asarray(neighbor_idx, dtype=np.int32)
    mask_f = np.asarray(neighbor_mask).astype(np.float32)
    W1 = np.asarray(W1, dtype=np.float32)
    W2 = np.asarray(W2, dtype=np.float32)
    W3 = np.asarray(W3, dtype=np.float32)
    b1 = np.asarray(b1, dtype=np.float32)
    b2 = np.asarray(b2, dtype=np.float32)
    b3 = np.asarray(b3, dtype=np.float32)

    bfc = lambda x: np.ascontiguousarray(x).astype(ml_dtypes.bfloat16)
    # w[p, h*256 + j*128 + jj] = W[h*128+p, j*128+jj]
    w_shared = {
        "w1": bfc(W1.reshape(2, 128, 256).transpose(1, 0, 2).reshape(128, 512)),
        "w2": bfc(W2.reshape(2, 128, 256).transpose(1, 0, 2).reshape(128, 512)),
        "w3": bfc(W3.reshape(2, 128).T),
        "b1c": np.ascontiguousarray(b1.reshape(2, 128).T).astype(np.float32),
        "b2c": np.ascontiguousarray(b2.reshape(2, 128).T).astype(np.float32),
        "b3v": np.full((1, 1), float(b3.reshape(-1)[0]), dtype=np.float32),
    }

    in_maps = [
        _prep_core_inputs(c, pool_f32, neighbor_idx, mask_f, w_shared)
        for c in range(N_CORES)
    ]
    res = run_bass_kernel_spmd(nc, in_maps, core_ids=list(range(N_CORES)))
    out = np.concatenate([res.results[c]["out"] for c in range(N_CORES)], axis=0)
    return out.astype(np.float32)


# revision 16
# speedup vs baseline: 504.3398x; 504.3398x over previous
"""Trainium2 Bass kernel for nn_AugmentWithTrace (gnn_message_passing).

Reference computation:
    g = trace_pool[neighbor_idx]                       # [T, K, D] gather
    s = MLP3(g)                                        # per-row scores
    attn = masked_softmax_k(s)                         # over K=8 neighbors
    out = einsum('tk,tkd->td', attn, g)                # [T, D]

Sharding: data-parallel over T across 8 NeuronCores; trace_pool and MLP
weights replicated. Per core: 4096 tokens = 32 chunks of 128 tokens.

Per-chunk dataflow (per core):
  - 8 indirect 128-row gathers (one per neighbor slot k); token t of the
    chunk lands on partition t, neighbor k in columns [k*256, (k+1)*256).
  - DVE cast to bf16, 16 PE transposes -> XT [128, 2048] bf16 where
    XT[p, h*1024 + k*128 + c] = g[token c, neighbor k, feature h*128+p].
  - 3-layer MLP in bf16 on the tensor engine (f32 PSUM accumulate), ReLU+bias
    fused into scalar-engine activation epilogues; h-major slot layout so all
    matmul rhs operands are contiguous [128, 512] slices.
  - scores [1, 1024] transposed to [128 tokens, 8 k] via tiny K=1 matmuls.
  - no-max softmax: exp on ACT (from SBUF; exp-from-PSUM kills the device),
    exact-zero masking, sum, reciprocal, scale on DVE (standard ISA ops only:
    the extended InstTensorTensorReduce also kills the device).
  - weighted sum: 8 fused multiply-add DVE ops against the f32 G tiles.
"""

import sys

if "/opt/trn_rl_repo" not in sys.path:
    sys.path.insert(0, "/opt/trn_rl_repo")

import numpy as np
import ml_dtypes

T, K, D, N_POOL = 32768, 8, 256, 131072
N_CORES = 8
T_LOC = T // N_CORES          # 4096 tokens per core
N_CHUNK = T_LOC // 128        # 32 chunks of 128 tokens

_CACHE = {}


def _build_kernel(b3_val=0.0, repeat=1, stage=9):
    import concourse.bass as bass
    import concourse.bacc as bacc
    import concourse.mybir as mybir
    import concourse.tile as tile
    from concourse.masks import make_identity

    f32 = mybir.dt.float32
    bf16 = mybir.dt.bfloat16
    i32 = mybir.dt.int32

    nc = bacc.Bacc("TRN2", target_bir_lowering=False, debug=False,
                   num_devices=N_CORES)

    pool_d = nc.declare_dram_parameter("pool", [N_POOL, D], f32, isOutput=False)
    idx_d = nc.declare_dram_parameter("idx", [128, N_CHUNK * K], i32, isOutput=False)
    maskc_d = nc.declare_dram_parameter("maskc", [128, N_CHUNK * K], f32, isOutput=False)
    w1_d = nc.declare_dram_parameter("w1", [128, 512], bf16, isOutput=False)
    w2_d = nc.declare_dram_parameter("w2", [128, 512], bf16, isOutput=False)
    w3_d = nc.declare_dram_parameter("w3", [128, 2], bf16, isOutput=False)
    b1_d = nc.declare_dram_parameter("b1c", [128, 2], f32, isOutput=False)
    b2_d = nc.declare_dram_parameter("b2c", [128, 2], f32, isOutput=False)
    out_d = nc.declare_dram_parameter("out", [T_LOC, D], f32, isOutput=True)

    with tile.TileContext(nc) as tc:
        with (
            tc.tile_pool(name="const", bufs=1) as cp,
            tc.tile_pool(name="gat", bufs=6) as gp,
            tc.tile_pool(name="gbf", bufs=3) as gbp,
            tc.tile_pool(name="xt", bufs=3) as xp,
            tc.tile_pool(name="act", bufs=3) as hp,
            tc.tile_pool(name="sml", bufs=3) as sp,
            tc.tile_pool(name="outp", bufs=3) as op_,
            tc.tile_pool(name="pmm", bufs=2, space="PSUM") as pmm,
            tc.tile_pool(name="ptp", bufs=2, space="PSUM") as ptp_pool,
            tc.tile_pool(name="psmall", bufs=2, space="PSUM") as psm,
        ):
            # ---- constants loaded once ----
            idx_t = cp.tile([128, N_CHUNK * K], i32)
            nc.sync.dma_start(out=idx_t[:], in_=idx_d[:])
            maskc_t = cp.tile([128, N_CHUNK * K], f32)
            nc.sync.dma_start(out=maskc_t[:], in_=maskc_d[:])
            w1_t = cp.tile([128, 512], bf16)
            nc.sync.dma_start(out=w1_t[:], in_=w1_d[:])
            w2_t = cp.tile([128, 512], bf16)
            nc.sync.dma_start(out=w2_t[:], in_=w2_d[:])
            w3_t = cp.tile([128, 2], bf16)
            nc.sync.dma_start(out=w3_t[:], in_=w3_d[:])
            b1_t = cp.tile([128, 2], f32)
            nc.sync.dma_start(out=b1_t[:], in_=b1_d[:])
            b2_t = cp.tile([128, 2], f32)
            nc.sync.dma_start(out=b2_t[:], in_=b2_d[:])
            ident = cp.tile([128, 128], bf16)
            make_identity(nc, ident[:])
            ones1 = cp.tile([1, 1], bf16)
            nc.vector.memset(ones1[:], 1.0)

            relu = mybir.ActivationFunctionType.Relu
            expf = mybir.ActivationFunctionType.Exp

            for rep in range(repeat):
              for c in range(N_CHUNK):
                # ---- gather: 8 indirect 128-row calls ----
                g_t = gp.tile([128, K * D], f32, name=f"g{rep}_{c}", tag="g")
                for k in range(K):
                    nc.gpsimd.indirect_dma_start(
                        out=g_t[:, k * D:(k + 1) * D],
                        out_offset=None,
                        in_=pool_d[:],
                        in_offset=bass.IndirectOffsetOnAxis(
                            ap=idx_t[:, c * K + k:c * K + k + 1], axis=0),
                    )
                if stage == 1:
                    nc.sync.dma_start(out=out_d[c * 128:(c + 1) * 128, :],
                                      in_=g_t[:, 0:D])
                    continue

                # ---- bf16 cast ----
                gb_t = gbp.tile([128, K * D], bf16, name=f"gb{rep}_{c}", tag="gb")
                nc.vector.tensor_copy(out=gb_t[:], in_=g_t[:])
                if stage == 15:
                    nc.sync.dma_start(out=out_d[c * 128:(c + 1) * 128, :],
                                      in_=g_t[:, 0:D])
                    continue

                # ---- PE transposes -> XT (h-major slots) ----
                xt_t = xp.tile([128, 2 * 8 * 128], bf16, name=f"xt{rep}_{c}", tag="xt")
                for h in range(2):
                    tp_t = ptp_pool.tile([128, 1024], bf16, name=f"tp{rep}_{c}{h}",
                                         tag="tp")
                    tp = tp_t[:]
                    for k in range(K):
                        nc.tensor.transpose(
                            out=tp[:, k * 128:(k + 1) * 128],
                            in_=gb_t[:, k * D + h * 128:k * D + (h + 1) * 128],
                            identity=ident[:])
                    nc.scalar.copy(
                        out=xt_t[:, h * 1024:(h + 1) * 1024], in_=tp[:])
                if stage == 2:
                    acc_t = op_.tile([128, D], f32, name=f"acc{rep}_{c}", tag="acc")
                    nc.vector.tensor_copy(out=acc_t[:], in_=xt_t[:, :D])
                    nc.sync.dma_start(out=out_d[c * 128:(c + 1) * 128, :], in_=acc_t[:])
                    continue

                # ---- layer 1 ----
                h1_t = hp.tile([128, 2 * 8 * 128], bf16, name=f"h1{rep}_{c}", tag="h1")
                for j in range(2):
                    ps1 = pmm.tile([128, 1024], f32, name=f"ps1{rep}_{c}{j}", tag="mm")
                    for n4 in range(2):
                        for h in range(2):
                            nc.tensor.matmul(
                                out=ps1[:, n4 * 512:(n4 + 1) * 512],
                                lhsT=w1_t[:, h * 256 + j * 128:h * 256 + (j + 1) * 128],
                                rhs=xt_t[:, h * 1024 + n4 * 512:h * 1024 + (n4 + 1) * 512],
                                start=(h == 0), stop=(h == 1),
                            )
                    nc.scalar.activation(
                        out=h1_t[:, j * 1024:(j + 1) * 1024],
                        in_=ps1[:],
                        func=relu, bias=b1_t[:, j:j + 1], scale=1.0)
                if stage == 3:
                    acc_t = op_.tile([128, D], f32, name=f"acc{rep}_{c}", tag="acc")
                    nc.vector.tensor_copy(out=acc_t[:], in_=h1_t[:, :D])
                    nc.sync.dma_start(out=out_d[c * 128:(c + 1) * 128, :], in_=acc_t[:])
                    continue

                # ---- layer 2 ----
                h2_t = hp.tile([128, 2 * 8 * 128], bf16, name=f"h2{rep}_{c}", tag="h2")
                for j in range(2):
                    ps2 = pmm.tile([128, 1024], f32, name=f"ps2{rep}_{c}{j}", tag="mm")
                    for n4 in range(2):
                        for h in range(2):
                            nc.tensor.matmul(
                                out=ps2[:, n4 * 512:(n4 + 1) * 512],
                                lhsT=w2_t[:, h * 256 + j * 128:h * 256 + (j + 1) * 128],
                                rhs=h1_t[:, h * 1024 + n4 * 512:h * 1024 + (n4 + 1) * 512],
                                start=(h == 0), stop=(h == 1),
                            )
                    nc.scalar.activation(
                        out=h2_t[:, j * 1024:(j + 1) * 1024],
                        in_=ps2[:],
                        func=relu, bias=b2_t[:, j:j + 1], scale=1.0)

                # ---- layer 3: scores [1, 1024] ----
                ps_sa = psm.tile([1, 512], f32, name=f"pssa{rep}_{c}", tag="psml")
                ps_sb = psm.tile([1, 512], f32, name=f"pssb{rep}_{c}", tag="psml")
                for n4, ps_half in ((0, ps_sa), (1, ps_sb)):
                    for h in range(2):
                        nc.tensor.matmul(
                            out=ps_half[:],
                            lhsT=w3_t[:, h:h + 1],
                            rhs=h2_t[:, h * 1024 + n4 * 512:h * 1024 + (n4 + 1) * 512],
                            start=(h == 0), stop=(h == 1),
                        )
                s_t = sp.tile([1, 1024], bf16, name=f"s{rep}_{c}", tag="s")
                nc.vector.tensor_copy(out=s_t[:, :512], in_=ps_sa[:])
                nc.vector.tensor_copy(out=s_t[:, 512:], in_=ps_sb[:])
                if stage == 4:
                    acc_t = op_.tile([128, D], f32, name=f"acc{rep}_{c}", tag="acc")
                    nc.vector.tensor_copy(out=acc_t[:], in_=h2_t[:, :D])
                    nc.sync.dma_start(out=out_d[c * 128:(c + 1) * 128, :], in_=acc_t[:])
                    continue

                # ---- transpose scores to [128 tokens, 8 k] ----
                psc_t = psm.tile([128, K], f32, name=f"psc{rep}_{c}", tag="psml")
                for k in range(K):
                    nc.tensor.matmul(
                        out=psc_t[:, k:k + 1],
                        lhsT=s_t[:, k * 128:(k + 1) * 128],
                        rhs=ones1[:],
                        start=True, stop=True,
                    )

                # ---- softmax over k (no max-subtraction; exact-0 masking) ----
                sc_t = sp.tile([128, K], f32, name=f"sc{rep}_{c}", tag="scb")
                nc.vector.tensor_copy(out=sc_t[:], in_=psc_t[:])
                e_t = sp.tile([128, K], f32, name=f"e{rep}_{c}", tag="e")
                nc.scalar.activation(out=e_t[:], in_=sc_t[:], func=expf,
                                     bias=float(b3_val), scale=1.0)
                em_t = sp.tile([128, K], f32, name=f"em{rep}_{c}", tag="em")
                z_t = sp.tile([128, 1], f32, name=f"z{rep}_{c}", tag="z")
                nc.vector.tensor_tensor(
                    out=em_t[:], in0=e_t[:], in1=maskc_t[:, c * K:(c + 1) * K],
                    op=mybir.AluOpType.mult)
                nc.vector.reduce_sum(z_t[:], em_t[:], axis=mybir.AxisListType.X)
                nc.vector.tensor_scalar_add(z_t[:], z_t[:], 1e-30)
                r_t = sp.tile([128, 1], f32, name=f"r{rep}_{c}", tag="r")
                nc.vector.reciprocal(out=r_t[:], in_=z_t[:])
                a_t = sp.tile([128, K], f32, name=f"a{rep}_{c}", tag="a")
                nc.vector.tensor_scalar_mul(a_t[:], em_t[:], r_t[:])

                # ---- weighted sum over k ----
                acc_t = op_.tile([128, D], f32, name=f"acc{rep}_{c}", tag="acc")
                nc.vector.tensor_scalar_mul(
                    acc_t[:], g_t[:, 0:D], a_t[:, 0:1])
                for k in range(1, K):
                    nc.vector.scalar_tensor_tensor(
                        out=acc_t[:],
                        in0=g_t[:, k * D:(k + 1) * D],
                        scalar=a_t[:, k:k + 1],
                        in1=acc_t[:],
                        op0=mybir.AluOpType.mult,
                        op1=mybir.AluOpType.add)

                nc.sync.dma_start(out=out_d[c * 128:(c + 1) * 128, :],
                                  in_=acc_t[:])

    nc.compile()
    return nc


def _prep_core_inputs(c, pool_f32, neighbor_idx, mask_f, w_shared):
    t0 = c * T_LOC
    nidx = neighbor_idx[t0:t0 + T_LOC]                     # [T_LOC, K]
    # column (chunk*K + k), partition p -> token chunk*128+p, neighbor k
    idx_arr = np.ascontiguousarray(
        nidx.reshape(N_CHUNK, 128, K).transpose(1, 0, 2).reshape(128, N_CHUNK * K)
    ).astype(np.int32)
    mcol = np.ascontiguousarray(
        mask_f[t0:t0 + T_LOC].reshape(N_CHUNK, 128, K)
        .transpose(1, 0, 2).reshape(128, N_CHUNK * K)
    ).astype(np.float32)
    m = {"pool": pool_f32, "idx": idx_arr, "maskc": mcol}
    m.update(w_shared)
    return m


def kernel(trace_pool, neighbor_idx, neighbor_mask, W1, b1, W2, b2, W3, b3):
    b3_arr = np.asarray(b3, dtype=np.float32)
    b3_val = float(b3_arr.reshape(-1)[0])
    if _CACHE.get("b3_val") != b3_val:
        _CACHE["nc"] = _build_kernel(b3_val)
        _CACHE["b3_val"] = b3_val
    nc = _CACHE["nc"]

    from concourse.bass_utils import run_bass_kernel_spmd

    pool_f32 = np.ascontiguousarray(np.asarray(trace_pool, dtype=np.float32))
    neighbor_idx = np.asarray(neighbor_idx, dtype=np.int32)
    mask_f = np.asarray(neighbor_mask).astype(np.float32)
    W1 = np.asarray(W1, dtype=np.float32)
    W2 = np.asarray(W2, dtype=np.float32)
    W3 = np.asarray(W3, dtype=np.float32)
    b1 = np.asarray(b1, dtype=np.float32)
    b2 = np.asarray(b2, dtype=np.float32)

    bfc = lambda x: np.ascontiguousarray(x).astype(ml_dtypes.bfloat16)
    # w[p, h*256 + j*128 + jj] = W[h*128+p, j*128+jj]
    w_shared = {
        "w1": bfc(W1.reshape(2, 128, 256).transpose(1, 0, 2).reshape(128, 512)),
        "w2": bfc(W2.reshape(2, 128, 256).transpose(1, 0, 2).reshape(128, 512)),
        "w3": bfc(W3.reshape(2, 128).T),
        "b1c": np.ascontiguousarray(b1.reshape(2, 128).T).astype(np.float32),
        "b2c": np.ascontiguousarray(b2.reshape(2, 128).T).astype(np.float32),
    }

    in_maps = [
        _prep_core_inputs(c, pool_f32, neighbor_idx, mask_f, w_shared)
        for c in range(N_CORES)
    ]
    res = run_bass_kernel_spmd(nc, in_maps, core_ids=list(range(N_CORES)))
    out = np.concatenate([res.results[c]["out"] for c in range(N_CORES)], axis=0)
    return out.astype(np.float32)
